# revision 6
# baseline (speedup 1.0000x reference)
"""Multi-head attention kernel for Trainium2, sharded over 8 NeuronCores.

Full inputs q,k,v: [2, 16, 2048, 64] fp32. Heads (B*H = 32) are sharded 4 per
core; each core computes softmax(Q K^T / sqrt(d)) V for its heads with no
cross-core communication.

v2 design (vs v1):
  - PV matmul transposed: out[q, d+1] = P @ [V | 1] with P-chunk [128k, 128q]
    stationary and [V_j | 1] (65 cols) moving -> halves PV stream cycles and
    eliminates v1's output transposes + PSUM->SBUF copies entirely. Column 64
    accumulates the softmax denominator (ones-column trick).
  - exp split: ACT does exact Exp on most key-chunks; DVE handles DVE_JS
    chunks via a one-instruction Schraudolph fp16 bit-trick
    (i16 = A*s + B, bitcast fp16 ~= exp(s/8), |rel err| <= ~3% pre-softmax),
    which softmax normalization mostly cancels end-to-end.
  - Tight pipeline: score runs 2 steps ahead; input transposes for head h+1
    ride head h's j-stream; optional ldweights filler keeps the PE from
    idling so its DVFS p-state stays at 2.4 GHz (idle resets it to 1.2 GHz).
  - finalize: DVE reciprocal of the denominator column, multiply + direct
    DMA out. No transposes.

PSUM (8 banks): st 2bufs x 2 + out0 2bufs x 1 + out1 1buf x 1 + tr 1buf x 1.
"""

import math
import sys

sys.path.insert(0, "/opt/trn_rl_repo")

import numpy as np

import concourse.bass as bass
import concourse.mybir as mybir
import concourse.tile as tile
from concourse import bacc
from concourse.bass_utils import run_bass_kernel_spmd
from concourse.masks import make_identity

B, H, N, D = 2, 16, 2048, 64
NCORES = 8
HPC = (B * H) // NCORES  # 4 heads per core
SCALE = float(D) ** -0.5

F32 = mybir.dt.float32
F16 = mybir.dt.float16
I16 = mybir.dt.int16
EXP = mybir.ActivationFunctionType.Exp
MUL = mybir.AluOpType.mult
ADD = mybir.AluOpType.add

NJ = 16  # key chunks of 128
IB = 1024  # query-block width (score/exp step width)
NIB = N // IB  # 2

# j indices whose exp runs on DVE via the bit trick (rest: exact exp on ACT).
# Keep 6,7,14,15 on ACT so DVE is free for ride-copies (j7/j15) and the
# finalize reciprocals (j15).
DVE_JS = frozenset({1, 3, 9, 11})

# ldweights-based PE filler per step (each ~100ns, no side effects): bridges
# the PE's wait for exp(s) so the p-state ramp isn't reset by idle gaps.
FILLER_LDW = 2

# finalize multiply engine: walrus rejects gpsimd PSUM access, so DVE it is.
FIN_MUL_ON_GPSIMD = False

# Schraudolph fp16 exp: i16 = trunc(EXP_A * s + EXP_B); bitcast fp16.
# A = 2^10 * scale / ln2; B = 15*2^10 - 2^10*C_opt + 0.5 (truncation bias).
EXP_A = 1024.0 * SCALE / math.log(2.0)
EXP_B = 15360.0 - 1024.0 * 0.04304 + 0.5


def _emit(tc):
    nc = tc.nc
    q_d = nc.dram_tensor("q", [HPC, N, D], F32, kind="ExternalInput").ap()
    k_d = nc.dram_tensor("k", [HPC, N, D], F32, kind="ExternalInput").ap()
    v_d = nc.dram_tensor("v", [HPC, N, D], F32, kind="ExternalInput").ap()
    o_d = nc.dram_tensor("o", [HPC, N, D], F32, kind="ExternalOutput").ap()

    from contextlib import ExitStack

    with ExitStack() as ctx:
        const_pool = ctx.enter_context(tc.tile_pool(name="const", bufs=1))
        stg = ctx.enter_context(tc.tile_pool(name="stg", bufs=2))
        kqt_pool = ctx.enter_context(tc.tile_pool(name="kqt", bufs=2))
        pt_pool = ctx.enter_context(tc.tile_pool(name="pt", bufs=4))
        fin_pool = ctx.enter_context(tc.tile_pool(name="fin", bufs=3))
        ps = ctx.enter_context(tc.tile_pool(name="ps", bufs=1, space="PSUM"))

        ident = const_pool.tile([128, 128], F16)
        make_identity(nc, ident[:])

        # ---------- staging DMAs (gpsimd casting fp32->fp16) ----------
        kstg, qstg, vstg = {}, {}, {}

        def stage_k(h):
            s = stg.tile([128, NJ, D], F16, tag="kstg", name=f"kstg{h}")
            nc.gpsimd.dma_start(s[:], k_d[h].rearrange("(t p) d -> p t d", p=128))
            kstg[h] = s

        def stage_q(h):
            s = stg.tile([128, NJ, D], F16, tag="qstg", name=f"qstg{h}")
            nc.gpsimd.dma_start(s[:], q_d[h].rearrange("(t p) d -> p t d", p=128))
            qstg[h] = s

        def stage_v(h):
            s = stg.tile([128, NJ, D + 1], F16, tag="vstg", bufs=3, name=f"vstg{h}")
            nc.gpsimd.dma_start(
                s[:, :, 0:D], v_d[h].rearrange("(t p) d -> p t d", p=128)
            )
            nc.gpsimd.memset(s[:, :, D : D + 1], 1.0)
            vstg[h] = s

        # ---------- transposed q/k (SBUF [64, 2048], d on partitions) ----------
        # creation order = usage order so the 2-buf rotation aliases head h
        # with head h+2 (whose windows never overlap).
        kts = {h: kqt_pool.tile([D, N], F16, tag="kt", name=f"kt{h}") for h in range(HPC)}
        qts = {h: kqt_pool.tile([D, N], F16, tag="qt", name=f"qt{h}") for h in range(HPC)}

        # transpose groups: chunks c0..c1-1 of a staging tile -> dst columns.
        # tr PSUM slot is bufs=1 -> groups must be strictly sequential in
        # emission order (each group's first transpose waits the previous
        # group's PSUM->SBUF copy).
        def make_group(src_get, dst, c0, c1):
            state = {}

            def tp(c):
                if "tr" not in state:
                    state["tr"] = ps.tile(
                        [D, (c1 - c0) * 128], F16, tag="tr", bufs=1, name="tr"
                    )
                src = src_get()
                nc.tensor.transpose(
                    state["tr"][:, (c - c0) * 128 : (c - c0 + 1) * 128],
                    src[:, c, :],
                    ident[:],
                )
                if c == c1 - 1:
                    nc.vector.tensor_copy(dst[:, c0 * 128 : c1 * 128], state["tr"][:])

            return [(lambda c=c: tp(c)) for c in range(c0, c1)]

        # ---------- phase 2 bookkeeping ----------
        blocks = [(h, ib) for h in range(HPC) for ib in range(NIB)]
        steps = [(h, ib, j) for (h, ib) in blocks for j in range(NJ)]
        TOT = len(steps)

        st_tiles = {}
        pt_tiles = {}
        out_tiles = {}

        def emit_score(s):
            h, ib, j = steps[s]
            st = ps.tile([128, IB], F32, tag="st", bufs=2, name="st")
            st_tiles[s] = st
            for c in range(IB // 512):
                nc.tensor.matmul(
                    st[:, c * 512 : (c + 1) * 512],
                    kts[h][:, j * 128 : (j + 1) * 128],
                    qts[h][:, ib * IB + c * 512 : ib * IB + (c + 1) * 512],
                    start=True,
                    stop=True,
                )

        def emit_exp(s):
            h, ib, j = steps[s]
            st = st_tiles[s]
            pt = pt_pool.tile([128, IB], F16, tag="pt", name="pt")
            pt_tiles[s] = pt
            if j in DVE_JS:
                nc.vector.tensor_scalar(
                    pt[:].bitcast(I16), st[:], EXP_A, EXP_B, MUL, ADD
                )
            else:
                nc.scalar.activation(pt[:], st[:], EXP, scale=SCALE)

        def emit_pv(s):
            h, ib, j = steps[s]
            bi = blocks.index((h, ib))
            pt = pt_tiles[s]
            for half in range(2):
                if j == 0:
                    out_tiles[(bi, half)] = ps.tile(
                        [128, 4, D + 1],
                        F32,
                        tag=f"out{half}",
                        bufs=2 if half == 0 else 1,
                        name=f"out{half}",
                    )
                ot = out_tiles[(bi, half)]
                for c in range(4):
                    qc = half * 4 + c
                    # one accumulation group per PSUM bank: start on the
                    # first write only (other j=0 chunks lazy-zero on first
                    # touch), stop on the very last.
                    nc.tensor.matmul(
                        ot[:, c, :],
                        pt[:, qc * 128 : (qc + 1) * 128],
                        vstg[h][:, j, :],
                        start=(j == 0 and c == 0),
                        stop=(j == NJ - 1 and c == 3),
                        skip_group_check=True,
                    )
            del st_tiles[s]

        def emit_finalize(bi):
            h, ib = blocks[bi]
            for half in (1, 0):  # half 1 first: its psum slot is single-buffered
                ot = out_tiles.pop((bi, half))
                rcp = fin_pool.tile([128, 4, 1], F32, tag="rcp", name="rcp")
                nc.vector.reciprocal(rcp[:], ot[:, :, D : D + 1])
                fin = fin_pool.tile([128, 4, D], F32, tag="fin", name="fin")
                eng = nc.gpsimd if FIN_MUL_ON_GPSIMD else nc.vector
                eng.tensor_mul(
                    fin[:], ot[:, :, 0:D], rcp[:].broadcast_to([128, 4, D])
                )
                t0 = ib * (IB // 128) + half * 4
                nc.sync.dma_start(
                    o_d[h].rearrange("(t p) d -> p t d", p=128)[:, t0 : t0 + 4, :],
                    fin[:],
                )

        # ---------- ride + DMA schedules ----------
        # Groups run 2 transposes/step so each group's copy lands >=1.5 steps
        # before the next group's first transpose (tr slot is single-buffered).
        rides = [[] for _ in range(TOT)]

        def sched(base, acts, per_step=2):
            for i, a in enumerate(acts):
                rides[base + i // per_step].append(a)

        for h in range(HPC):
            w0 = (2 * h) * NJ
            w1 = (2 * h + 1) * NJ
            if h == 0:
                # q0 second half first (needed by window (0,1)).
                sched(w0 + 0, make_group(lambda: qstg[0], qts[0], 8, 12))
                sched(w0 + 2, make_group(lambda: qstg[0], qts[0], 12, 16))
                k1g1, k1g2 = 4, 9
            else:
                k1g1, k1g2 = 0, 5
            if h + 1 < HPC:
                kt, qt = kts[h + 1], qts[h + 1]
                sched(w0 + k1g1, make_group(lambda h=h: kstg[h + 1], kt, 0, 8))
                sched(w0 + k1g2, make_group(lambda h=h: kstg[h + 1], kt, 8, 16))
                sched(w1 + 0, make_group(lambda h=h: qstg[h + 1], qt, 0, 8))
                sched(w1 + 5, make_group(lambda h=h: qstg[h + 1], qt, 8, 16))

        dma_sched = [[] for _ in range(TOT)]
        dma_sched[0].append(lambda: stage_v(1))
        for h in range(1, HPC - 1):
            w0 = (2 * h - 2) * NJ  # issue head h+1's tensors a window early
            dma_sched[w0 + 2].append(lambda h=h: stage_k(h + 1))
            dma_sched[w0 + NJ + 2].append(lambda h=h: stage_q(h + 1))
            dma_sched[w0 + NJ + 10].append(lambda h=h: stage_v(h + 1))

        # ---------- phase 1: head 0 warm-up ----------
        stage_k(0)
        stage_q(0)
        stage_v(0)
        for acts in (
            make_group(lambda: kstg[0], kts[0], 0, 8),
            make_group(lambda: kstg[0], kts[0], 8, 16),
            make_group(lambda: qstg[0], qts[0], 0, 8),
        ):
            for a in acts:
                a()
        stage_k(1)
        stage_q(1)

        # ---------- phase 2: main loop ----------
        emit_score(0)
        emit_score(1)
        for s in range(TOT):
            h, ib, j = steps[s]
            for a in dma_sched[s]:
                a()
            emit_exp(s)
            for _ in range(FILLER_LDW):
                nc.tensor.ldweights(ident[:])
            if s + 2 < TOT:
                emit_score(s + 2)
            for a in rides[s]:
                a()
            emit_pv(s)
            if j == NJ - 1:
                emit_finalize(blocks.index((h, ib)))


_CACHE = {}


def _build():
    if "nc" in _CACHE:
        return _CACHE["nc"]
    nc = bacc.Bacc("TRN2", target_bir_lowering=False, debug=False, num_devices=NCORES)
    with tile.TileContext(nc) as tc:
        _emit(tc)
    nc.compile()
    _CACHE["nc"] = nc
    return nc


def run(q, k, v, trace=False, **spmd_kwargs):
    nc = _build()
    qf = np.ascontiguousarray(np.asarray(q, dtype=np.float32).reshape(B * H, N, D))
    kf = np.ascontiguousarray(np.asarray(k, dtype=np.float32).reshape(B * H, N, D))
    vf = np.ascontiguousarray(np.asarray(v, dtype=np.float32).reshape(B * H, N, D))
    in_maps = [
        {
            "q": qf[c * HPC : (c + 1) * HPC],
            "k": kf[c * HPC : (c + 1) * HPC],
            "v": vf[c * HPC : (c + 1) * HPC],
        }
        for c in range(NCORES)
    ]
    res = run_bass_kernel_spmd(
        nc, in_maps, list(range(NCORES)), trace=trace, **spmd_kwargs
    )
    out = np.concatenate([res.results[c]["o"] for c in range(NCORES)], axis=0)
    return out.reshape(B, H, N, D).astype(np.float32), res


def kernel(q, k, v):
    out, _ = run(q, k, v)
    return out
